# revision 28
# baseline (speedup 1.0000x reference)
"""Multi-head attention (B=2, S=2048, D=1024, H=16) on 8 trn2 NeuronCores.

Sharding: 2 groups of 4 cores; group b owns batch b, core (group rank r)
owns heads [4r:4r+4] (channels [256r:256r+256]). Each core loads only its
batch's x (8.4 MB), transposes it via the PE, projects q/k/v for its 4
heads, and runs attention. The output projection is re-sharded by sequence
rows via one 8-way AllToAll: destination core d takes rows [256d:256d+256)
of BOTH batches, so every source core sends it a [256ch x 256row] chunk.

Matmuls run in float32r (full PE rate at moving-dim >= 256). Measured on
HW: mixing K=64 and K=128 matmuls in the PE stream degrades every matmul
to ~724ns (vs 227ns uniform), so the per-head d_k=64 scores contraction is
zero-padded to K=128: kT is stored per-head with the sibling head's 64
partition rows zeroed, letting the head-pair's full 128-row qT be used as
the moving operand (the zero rows kill the cross-head terms).

v is stored per head-pair j-tile as [v_even | ones | v_odd] (192 cols):
head h's attn@v lhsT is the contiguous 128-col slice at 64*(h%2), so the
attn@v output rows hold {even head: raw 0:64, denom 64:128} and {odd head:
denom 0:64, raw 64:128} — one shared ones block replicates the softmax
denominator onto every partition for the normalize step.
"""

import numpy as np

import concourse.bass as bass
import concourse.mybir as mybir
import concourse.tile as tile
from concourse import bacc
from concourse.masks import make_identity
from concourse.bass_utils import run_bass_kernel_spmd

# problem constants (hardcoded per harness contract)
B, S, D = 2, 2048, 1024
H, DK = 16, 64
NCORES = 8
GPB = NCORES // B          # cores per batch group = 4
HPC = H // NCORES * B      # heads per core = 4
NPAIR = HPC // 2           # head pairs per core = 2
CS = HPC * DK              # per-core channel slice = 256
TCH = 512                  # stage-A t-chunk
NTCH = S // TCH            # 4 chunks (one batch per core)
IB = 512                   # stage-B i-chunk
RB = 256                   # output rows per (core, batch)
P = 128
F32 = mybir.dt.float32
F32R = mybir.dt.float32r
AF = mybir.ActivationFunctionType
ALU = mybir.AluOpType


def build_nc():
    nc = bacc.Bacc("TRN2", target_bir_lowering=False, debug=False, num_devices=NCORES)

    x = nc.dram_tensor("x", [S, D], F32, kind="ExternalInput")
    wqT = nc.dram_tensor("wqT", [D, CS], F32R, kind="ExternalInput")
    wkT = nc.dram_tensor("wkT", [D, CS], F32R, kind="ExternalInput")
    wvT = nc.dram_tensor("wvT", [D, CS], F32R, kind="ExternalInput")
    woT = nc.dram_tensor("woT", [D, D], F32R, kind="ExternalInput")
    bq = nc.dram_tensor("bq", [P, NPAIR], F32, kind="ExternalInput")
    bk = nc.dram_tensor("bk", [P, NPAIR], F32, kind="ExternalInput")
    bv = nc.dram_tensor("bv", [P, NPAIR], F32, kind="ExternalInput")
    bo = nc.dram_tensor("bo", [1, D], F32R, kind="ExternalInput")
    y = nc.dram_tensor("y", [2 * RB, D], F32, kind="ExternalOutput")

    with tile.TileContext(nc) as tc:
        with (
            tc.tile_pool(name="const", bufs=1) as cpool,
            tc.tile_pool(name="persist", bufs=1) as ppool,
            tc.tile_pool(name="dram", bufs=1, space="DRAM") as dpool,
        ):
            ident = cpool.tile([P, P], F32)
            make_identity(nc, ident[:])

            bq_sb = cpool.tile([P, NPAIR], F32)
            bk_sb = cpool.tile([P, NPAIR], F32)
            bv_sb = cpool.tile([P, NPAIR], F32)
            bo_sb = cpool.tile([1, D], F32R)
            nc.gpsimd.dma_start(bq_sb[:], bq[:])
            nc.gpsimd.dma_start(bk_sb[:], bk[:])
            nc.gpsimd.dma_start(bv_sb[:], bv[:])
            nc.gpsimd.dma_start(bo_sb[:], bo[:])

            ones32 = cpool.tile([P, 512], F32)
            nc.vector.memset(ones32[:], 1.0)
            zeros32 = cpool.tile([P, 512], F32)
            nc.vector.memset(zeros32[:], 0.0)
            ones_row = cpool.tile([1, P], F32R)
            nc.vector.tensor_copy(ones_row[:], ones32[0:1, 0:1].to_broadcast([1, P]))

            # bo broadcast to all partitions via two K=1 matmuls, done once
            # up-front so no K=1 matmul pollutes the K=128 streams later
            bo_full = cpool.tile([P, D], F32)
            with tc.tile_pool(name="psbo", bufs=1, space="PSUM") as psbo:
                for nch in range(D // 512):
                    pb = psbo.tile([P, 512], F32, tag="pb")
                    nc.tensor.matmul(
                        pb[:], ones_row[:], bo_sb[:, nch * 512:(nch + 1) * 512],
                        start=True, stop=True,
                    )
                    nc.vector.tensor_copy(bo_full[:, nch * 512:(nch + 1) * 512], pb[:])

            # persistent activations: one tile per head-pair / per head
            qTq = [ppool.tile([P, S], F32R, name=f"qT{pp_}") for pp_ in range(NPAIR)]
            kTh = [ppool.tile([P, S], F32R, name=f"kTh{h}") for h in range(HPC)]
            for h in range(HPC):
                z0 = (1 - h % 2) * DK  # zero rows: even head -> 64:128, odd -> 0:64
                nc.vector.tensor_copy(
                    kTh[h][z0:z0 + DK, :],
                    zeros32[z0:z0 + DK, None, :].to_broadcast([DK, S // 512, 512]),
                )
            # v per head-pair (see module docstring)
            v_sbp = [ppool.tile([P, S // P, 3 * DK], F32R, name=f"v{pp_}")
                     for pp_ in range(NPAIR)]
            for pp_ in range(NPAIR):
                nc.vector.tensor_copy(
                    v_sbp[pp_][:, :, DK:2 * DK],
                    ones32[:, None, 0:DK].to_broadcast([P, S // P, DK]),
                )

            # ---- stage A: x transpose + q/k/v projections ----
            with (
                tc.tile_pool(name="aw", bufs=1) as awpool,
                tc.tile_pool(name="stageA", bufs=2) as apool,
                tc.tile_pool(name="psA", bufs=2, space="PSUM") as psA,
                tc.tile_pool(name="psP", bufs=3, space="PSUM") as psP,
            ):
                # weights/biases go over the gpsimd SWDGE queue so they do
                # not delay the x stream on the sync HWDGE queue
                wq_sb = awpool.tile([P, 8, CS], F32R)
                wk_sb = awpool.tile([P, 8, CS], F32R)
                wv_sb = awpool.tile([P, 8, CS], F32R)
                nc.gpsimd.dma_start(wq_sb[:], wqT[:].rearrange("(o p) c -> p o c", p=P))
                nc.gpsimd.dma_start(wk_sb[:], wkT[:].rearrange("(o p) c -> p o c", p=P))
                nc.gpsimd.dma_start(wv_sb[:], wvT[:].rearrange("(o p) c -> p o c", p=P))
                vT = [awpool.tile([P, S], F32, name=f"vT{pp_}")
                      for pp_ in range(NPAIR)]
                for te in range(NTCH):
                    x_e = apool.tile([P, TCH // P, D], F32, tag="x_e")
                    if te == 0:
                        # split the first load so transposes start sooner
                        for hh in range(4):
                            r0 = hh * (TCH // 4)
                            nc.sync.dma_start(
                                x_e[:, hh:hh + 1, :],
                                x[r0:r0 + TCH // 4, :].rearrange(
                                    "(tt p) d -> p tt d", p=P
                                ),
                            )
                    else:
                        nc.sync.dma_start(
                            x_e[:],
                            x[te * TCH:(te + 1) * TCH, :].rearrange(
                                "(tt p) d -> p tt d", p=P
                            ),
                        )
                    xT_e = apool.tile([P, 8, TCH], F32R, tag="xT_e")
                    for dd in range(8):
                        ps = psA.tile([P, TCH], F32, tag="trps")
                        for tt in range(TCH // P):
                            nc.tensor.transpose(
                                ps[:, tt * P:(tt + 1) * P],
                                x_e[:, tt, dd * P:(dd + 1) * P],
                                ident[:],
                            )
                        nc.vector.tensor_copy(xT_e[:, dd, :], ps[:])
                    sl = slice(te * TCH, (te + 1) * TCH)
                    for proj, (w_sb, b_sb) in enumerate(
                        ((wq_sb, bq_sb), (wk_sb, bk_sb), (wv_sb, bv_sb))
                    ):
                        for ct in range(NPAIR):
                            cs = slice(ct * P, (ct + 1) * P)
                            pp = psP.tile([P, TCH], F32, tag="projps")
                            for dd in range(8):
                                nc.tensor.matmul(
                                    pp[:],
                                    w_sb[:, dd, cs],
                                    xT_e[:, dd, :],
                                    start=(dd == 0),
                                    stop=(dd == 7),
                                )
                            if proj == 0:
                                nc.vector.tensor_tensor(
                                    qTq[ct][:, sl], pp[:],
                                    b_sb[:, ct:ct + 1].to_broadcast([P, TCH]),
                                    ALU.add,
                                )
                            elif proj == 1:
                                for hh in range(2):
                                    hs = slice(hh * DK, (hh + 1) * DK)
                                    nc.vector.tensor_tensor(
                                        kTh[2 * ct + hh][hs, sl],
                                        pp[hs, :],
                                        b_sb[hs, ct:ct + 1
                                             ].to_broadcast([DK, TCH]),
                                        ALU.add,
                                    )
                            else:
                                nc.vector.tensor_tensor(
                                    vT[ct][:, sl], pp[:],
                                    b_sb[:, ct:ct + 1].to_broadcast([P, TCH]),
                                    ALU.add,
                                )
                # vT [c, t] -> v natural [t, c] split around the ones block
                for pp_ in range(NPAIR):
                    for tt in range(S // P):
                        psv = psA.tile([P, P], F32, tag="vtr")
                        nc.tensor.transpose(
                            psv[:], vT[pp_][:, tt * P:(tt + 1) * P], ident[:])
                        nc.vector.tensor_copy(
                            v_sbp[pp_][:, tt, 0:DK], psv[:, 0:DK])
                        nc.vector.tensor_copy(
                            v_sbp[pp_][:, tt, 2 * DK:3 * DK], psv[:, DK:2 * DK])

            # ---- stage C tiles allocated now (reuse stage A's space);
            # the 4 MB wo load overlaps stage B compute on the sync queue
            with (
                tc.tile_pool(name="stageC", bufs=1) as c2pool,
                tc.tile_pool(name="yout", bufs=2) as ypool,
            ):
                wo_sb = c2pool.tile([P, 8, D], F32R)
                nc.sync.dma_start(wo_sb[:], woT[:].rearrange("(o p) n -> p o n", p=P))
                h_sb = c2pool.tile([P, 2, 8, RB], F32R)

                # ---- stage B: attention per output i-chunk ----
                a2a_in = dpool.tile([NCORES, CS, RB], F32R)
                a2a_out = dpool.tile([NCORES, CS, RB], F32R)
                with (
                    tc.tile_pool(name="et", bufs=7) as etpool,
                    tc.tile_pool(name="ob", bufs=3) as obpool,
                    tc.tile_pool(name="psS", bufs=1, space="PSUM") as psS,
                    tc.tile_pool(name="psAV", bufs=1, space="PSUM") as psAV,
                ):
                    for g in range(S // IB):
                        i0 = g * IB
                        av_ps = [
                            psAV.tile([P, IB], F32, tag=f"av{h}", name=f"av{h}")
                            for h in range(HPC)
                        ]
                        for jt in range(S // P):
                            j0 = jt * P
                            sps = [psS.tile([P, 2 * IB], F32, tag=f"s{pp_}",
                                            name=f"s{pp_}")
                                   for pp_ in range(NPAIR)]
                            for h in range(HPC):
                                nc.tensor.matmul(
                                    sps[h // 2][:, (h % 2) * IB:(h % 2 + 1) * IB],
                                    kTh[h][:, j0:j0 + P],
                                    qTq[h // 2][:, i0:i0 + IB],
                                    start=True,
                                    stop=True,
                                )
                            ets = []
                            for pp_ in range(NPAIR):
                                et = etpool.tile([P, 2 * IB], F32R, tag="et")
                                nc.scalar.activation(
                                    et[:], sps[pp_][:], AF.Exp, scale=0.125)
                                ets.append(et)
                            for h in range(HPC):
                                nc.tensor.matmul(
                                    av_ps[h][:],
                                    v_sbp[h // 2][:, jt,
                                                  (h % 2) * DK:
                                                  (h % 2) * DK + 2 * DK],
                                    ets[h // 2][:, (h % 2) * IB:(h % 2 + 1) * IB],
                                    start=(jt == 0),
                                    stop=(jt == S // P - 1),
                                )
                        for pp_ in range(NPAIR):
                            # merge the pair: raw rows into A, denom rows into
                            # D (even head: raw 0:64/den 64:128; odd head
                            # flipped), one reciprocal + multiply per pair
                            e, o = 2 * pp_, 2 * pp_ + 1
                            rawA = obpool.tile([P, IB], F32, tag="rawA")
                            nc.vector.tensor_copy(rawA[0:DK, :], av_ps[e][0:DK, :])
                            nc.vector.tensor_copy(rawA[DK:, :], av_ps[o][DK:, :])
                            den = obpool.tile([P, IB], F32, tag="den")
                            nc.vector.tensor_copy(den[0:DK, :], av_ps[e][DK:, :])
                            nc.vector.tensor_copy(den[DK:, :], av_ps[o][0:DK, :])
                            rec = obpool.tile([P, IB], F32, tag="rec")
                            nc.vector.reciprocal(rec[:], den[:])
                            onrm = obpool.tile([P, IB], F32, tag="onrm")
                            nc.vector.tensor_tensor(
                                onrm[:], rawA[:], rec[:], ALU.mult,
                            )
                            # chunk rows [512g:512g+512) split to dests 2g,2g+1
                            for dd2 in range(2):
                                nc.sync.dma_start(
                                    a2a_in[2 * g + dd2,
                                           pp_ * P:(pp_ + 1) * P, :],
                                    onrm[:, dd2 * RB:(dd2 + 1) * RB
                                         ].bitcast(F32R),
                                )

                nc.gpsimd.collective_compute(
                    "AllToAll",
                    ALU.bypass,
                    replica_groups=[list(range(NCORES))],
                    ins=[a2a_in.opt()],
                    outs=[a2a_out.opt()],
                )

                # ---- stage C: output projection for my 2x256 rows ----
                # received chunk s (s<4: batch0, s>=4: batch1) holds channels
                # [256s mod 1024 ...] of my row block; channel sub-tile o of
                # half hb lives at a2a_out[hb*4 + o//2, (o%2)*128:...]
                with tc.tile_pool(name="psY", bufs=2, space="PSUM") as psY:
                    for hb in range(2):
                        for o in range(8):
                            nc.sync.dma_start(
                                h_sb[:, hb, o, :],
                                a2a_out[hb * GPB + o // 2,
                                        (o % 2) * P:(o % 2 + 1) * P, :],
                            )
                    for hb in range(2):
                        for it in range(RB // P):
                            y_sb = ypool.tile([P, D], F32, tag="y")
                            for nch in range(D // 512):
                                py = psY.tile([P, 512], F32, tag="py")
                                for o in range(8):
                                    nc.tensor.matmul(
                                        py[:],
                                        h_sb[:, hb, o, it * P:(it + 1) * P],
                                        wo_sb[:, o, nch * 512:(nch + 1) * 512],
                                        start=(o == 0),
                                        stop=(o == 7),
                                    )
                                nc.vector.tensor_tensor(
                                    y_sb[:, nch * 512:(nch + 1) * 512],
                                    py[:],
                                    bo_full[:, nch * 512:(nch + 1) * 512],
                                    ALU.add,
                                )
                            r0 = hb * RB + it * P
                            nc.sync.dma_start(y[r0:r0 + P, :], y_sb[:])

    nc.compile()
    return nc


_NC = None


def _get_nc():
    global _NC
    if _NC is None:
        _NC = build_nc()
    return _NC


def _make_in_maps(x, Wq, bq, Wk, bk, Wv, bv, Wo, bo):
    xf = np.asarray(x, np.float32).reshape(B, S, D)
    woT = np.ascontiguousarray(np.asarray(Wo, np.float32).T)
    bo_r = np.ascontiguousarray(np.asarray(bo, np.float32).reshape(1, D))
    Wq = np.asarray(Wq, np.float32)
    Wk = np.asarray(Wk, np.float32)
    Wv = np.asarray(Wv, np.float32)
    in_maps = []
    for c in range(NCORES):
        b = c // GPB
        r = c % GPB
        sl = slice(r * CS, (r + 1) * CS)
        in_maps.append({
            "x": np.ascontiguousarray(xf[b]),
            "wqT": np.ascontiguousarray(Wq[sl, :].T),
            "wkT": np.ascontiguousarray(Wk[sl, :].T),
            "wvT": np.ascontiguousarray(Wv[sl, :].T),
            "woT": woT,
            "bq": np.ascontiguousarray(
                np.asarray(bq, np.float32)[sl].reshape(NPAIR, P).T),
            "bk": np.ascontiguousarray(
                np.asarray(bk, np.float32)[sl].reshape(NPAIR, P).T),
            "bv": np.ascontiguousarray(
                np.asarray(bv, np.float32)[sl].reshape(NPAIR, P).T),
            "bo": bo_r,
        })
    return in_maps


def _assemble(results):
    yout = np.empty((B, S, D), np.float32)
    for d in range(NCORES):
        rows = slice(d * RB, (d + 1) * RB)
        yout[0, rows, :] = results[d]["y"][0:RB]
        yout[1, rows, :] = results[d]["y"][RB:2 * RB]
    return yout


def run_traced(trace=False, **inputs):
    """Run and return (output, BassKernelResults) — used by test.py."""
    nc = _get_nc()
    res = run_bass_kernel_spmd(
        nc, _make_in_maps(**inputs), core_ids=list(range(NCORES)), trace=trace
    )
    return _assemble(res.results), res


def kernel(**inputs) -> np.ndarray:
    out, _ = run_traced(trace=False, **inputs)
    return out


# revision 29
# speedup vs baseline: 1.1186x; 1.1186x over previous
"""Multi-head attention (B=2, S=2048, D=1024, H=16) on 8 trn2 NeuronCores.

Sharding: 2 groups of 4 cores; group b owns batch b, core (group rank r)
owns heads [4r:4r+4] (channels [256r:256r+256]). Each core loads only its
batch's x (8.4 MB), transposes it via the PE, projects q/k/v for its 4
heads, and runs attention. The output projection is re-sharded by sequence
rows via one 8-way AllToAll: destination core d takes rows [256d:256d+256)
of BOTH batches, so every source core sends it a [256ch x 256row] chunk.

Matmuls run in float32r (full PE rate at moving-dim >= 256). Measured on
HW: mixing K=64 and K=128 matmuls in the PE stream degrades every matmul
to ~724ns (vs 227ns uniform), so the per-head d_k=64 scores contraction is
zero-padded to K=128: kT is stored per-head with the sibling head's 64
partition rows zeroed, letting the head-pair's full 128-row qT be used as
the moving operand (the zero rows kill the cross-head terms).

v is stored per head-pair j-tile as [v_even | ones | v_odd] (192 cols):
head h's attn@v lhsT is the contiguous 128-col slice at 64*(h%2), so the
attn@v output rows hold {even head: raw 0:64, denom 64:128} and {odd head:
denom 0:64, raw 64:128} — one shared ones block replicates the softmax
denominator onto every partition for the normalize step.
"""

import numpy as np

import concourse.bass as bass
import concourse.mybir as mybir
import concourse.tile as tile
from concourse import bacc
from concourse.masks import make_identity
from concourse.bass_utils import run_bass_kernel_spmd

# problem constants (hardcoded per harness contract)
B, S, D = 2, 2048, 1024
H, DK = 16, 64
NCORES = 8
GPB = NCORES // B          # cores per batch group = 4
HPC = H // NCORES * B      # heads per core = 4
NPAIR = HPC // 2           # head pairs per core = 2
CS = HPC * DK              # per-core channel slice = 256
TCH = 512                  # stage-A t-chunk
NTCH = S // TCH            # 4 chunks (one batch per core)
IB = 512                   # stage-B i-chunk
RB = 256                   # output rows per (core, batch)
P = 128
F32 = mybir.dt.float32
F32R = mybir.dt.float32r
AF = mybir.ActivationFunctionType
ALU = mybir.AluOpType


def build_nc():
    nc = bacc.Bacc("TRN2", target_bir_lowering=False, debug=False, num_devices=NCORES)

    x = nc.dram_tensor("x", [S, D], F32, kind="ExternalInput")
    wqT = nc.dram_tensor("wqT", [D, CS], F32R, kind="ExternalInput")
    wkT = nc.dram_tensor("wkT", [D, CS], F32R, kind="ExternalInput")
    wvT = nc.dram_tensor("wvT", [D, CS], F32R, kind="ExternalInput")
    woT = nc.dram_tensor("woT", [D, D], F32R, kind="ExternalInput")
    bq = nc.dram_tensor("bq", [P, NPAIR], F32, kind="ExternalInput")
    bk = nc.dram_tensor("bk", [P, NPAIR], F32, kind="ExternalInput")
    bv = nc.dram_tensor("bv", [P, NPAIR], F32, kind="ExternalInput")
    bo = nc.dram_tensor("bo", [1, D], F32R, kind="ExternalInput")
    y = nc.dram_tensor("y", [2 * RB, D], F32, kind="ExternalOutput")

    with tile.TileContext(nc) as tc:
        with (
            tc.tile_pool(name="const", bufs=1) as cpool,
            tc.tile_pool(name="persist", bufs=1) as ppool,
            tc.tile_pool(name="dram", bufs=1, space="DRAM") as dpool,
        ):
            ident = cpool.tile([P, P], F32)
            make_identity(nc, ident[:])

            bq_sb = cpool.tile([P, NPAIR], F32)
            bk_sb = cpool.tile([P, NPAIR], F32)
            bv_sb = cpool.tile([P, NPAIR], F32)
            bo_sb = cpool.tile([1, D], F32R)
            nc.gpsimd.dma_start(bq_sb[:], bq[:])
            nc.gpsimd.dma_start(bk_sb[:], bk[:])
            nc.gpsimd.dma_start(bv_sb[:], bv[:])
            nc.gpsimd.dma_start(bo_sb[:], bo[:])

            ones32 = cpool.tile([P, 512], F32)
            nc.vector.memset(ones32[:], 1.0)
            zeros32 = cpool.tile([P, 512], F32)
            nc.vector.memset(zeros32[:], 0.0)
            ones_row = cpool.tile([1, P], F32R)
            nc.vector.tensor_copy(ones_row[:], ones32[0:1, 0:1].to_broadcast([1, P]))

            # bo broadcast to all partitions via two K=1 matmuls, done once
            # up-front so no K=1 matmul pollutes the K=128 streams later
            bo_full = cpool.tile([P, D], F32)
            with tc.tile_pool(name="psbo", bufs=1, space="PSUM") as psbo:
                for nch in range(D // 512):
                    pb = psbo.tile([P, 512], F32, tag="pb")
                    nc.tensor.matmul(
                        pb[:], ones_row[:], bo_sb[:, nch * 512:(nch + 1) * 512],
                        start=True, stop=True,
                    )
                    nc.vector.tensor_copy(bo_full[:, nch * 512:(nch + 1) * 512], pb[:])

            # persistent activations: one tile per head-pair / per head
            qTq = [ppool.tile([P, S], F32R, name=f"qT{pp_}") for pp_ in range(NPAIR)]
            kTh = [ppool.tile([P, S], F32R, name=f"kTh{h}") for h in range(HPC)]
            for h in range(HPC):
                z0 = (1 - h % 2) * DK  # zero rows: even head -> 64:128, odd -> 0:64
                nc.vector.tensor_copy(
                    kTh[h][z0:z0 + DK, :],
                    zeros32[z0:z0 + DK, None, :].to_broadcast([DK, S // 512, 512]),
                )
            # v per head-pair (see module docstring)
            v_sbp = [ppool.tile([P, S // P, 3 * DK], F32R, name=f"v{pp_}")
                     for pp_ in range(NPAIR)]
            for pp_ in range(NPAIR):
                nc.vector.tensor_copy(
                    v_sbp[pp_][:, :, DK:2 * DK],
                    ones32[:, None, 0:DK].to_broadcast([P, S // P, DK]),
                )

            # ---- stage A: x transpose + q/k/v projections ----
            with (
                tc.tile_pool(name="aw", bufs=1) as awpool,
                tc.tile_pool(name="stageA", bufs=2) as apool,
                tc.tile_pool(name="psA", bufs=2, space="PSUM") as psA,
                tc.tile_pool(name="psP", bufs=3, space="PSUM") as psP,
            ):
                # weights/biases go over the gpsimd SWDGE queue so they do
                # not delay the x stream on the sync HWDGE queue
                wq_sb = awpool.tile([P, 8, CS], F32R)
                wk_sb = awpool.tile([P, 8, CS], F32R)
                wv_sb = awpool.tile([P, 8, CS], F32R)
                nc.gpsimd.dma_start(wq_sb[:], wqT[:].rearrange("(o p) c -> p o c", p=P))
                nc.gpsimd.dma_start(wk_sb[:], wkT[:].rearrange("(o p) c -> p o c", p=P))
                nc.gpsimd.dma_start(wv_sb[:], wvT[:].rearrange("(o p) c -> p o c", p=P))
                vT = [awpool.tile([P, S], F32, name=f"vT{pp_}")
                      for pp_ in range(NPAIR)]
                for te in range(NTCH):
                    x_e = apool.tile([P, TCH // P, D], F32, tag="x_e")
                    if te == 0:
                        # split the first load so transposes start sooner
                        for hh in range(4):
                            r0 = hh * (TCH // 4)
                            nc.sync.dma_start(
                                x_e[:, hh:hh + 1, :],
                                x[r0:r0 + TCH // 4, :].rearrange(
                                    "(tt p) d -> p tt d", p=P
                                ),
                            )
                    else:
                        nc.sync.dma_start(
                            x_e[:],
                            x[te * TCH:(te + 1) * TCH, :].rearrange(
                                "(tt p) d -> p tt d", p=P
                            ),
                        )
                    xT_e = apool.tile([P, 8, TCH], F32R, tag="xT_e")
                    for dd in range(8):
                        ps = psA.tile([P, TCH], F32, tag="trps")
                        for tt in range(TCH // P):
                            nc.tensor.transpose(
                                ps[:, tt * P:(tt + 1) * P],
                                x_e[:, tt, dd * P:(dd + 1) * P],
                                ident[:],
                            )
                        nc.vector.tensor_copy(xT_e[:, dd, :], ps[:])
                    sl = slice(te * TCH, (te + 1) * TCH)
                    for proj, (w_sb, b_sb) in enumerate(
                        ((wq_sb, bq_sb), (wk_sb, bk_sb), (wv_sb, bv_sb))
                    ):
                        for ct in range(NPAIR):
                            cs = slice(ct * P, (ct + 1) * P)
                            pp = psP.tile([P, TCH], F32, tag="projps")
                            for dd in range(8):
                                nc.tensor.matmul(
                                    pp[:],
                                    w_sb[:, dd, cs],
                                    xT_e[:, dd, :],
                                    start=(dd == 0),
                                    stop=(dd == 7),
                                )
                            if proj == 0:
                                nc.vector.tensor_tensor(
                                    qTq[ct][:, sl], pp[:],
                                    b_sb[:, ct:ct + 1].to_broadcast([P, TCH]),
                                    ALU.add,
                                )
                            elif proj == 1:
                                for hh in range(2):
                                    hs = slice(hh * DK, (hh + 1) * DK)
                                    nc.vector.tensor_tensor(
                                        kTh[2 * ct + hh][hs, sl],
                                        pp[hs, :],
                                        b_sb[hs, ct:ct + 1
                                             ].to_broadcast([DK, TCH]),
                                        ALU.add,
                                    )
                            else:
                                nc.vector.tensor_tensor(
                                    vT[ct][:, sl], pp[:],
                                    b_sb[:, ct:ct + 1].to_broadcast([P, TCH]),
                                    ALU.add,
                                )
                # vT [c, t] -> v natural [t, c] split around the ones block
                for pp_ in range(NPAIR):
                    for tt in range(S // P):
                        psv = psA.tile([P, P], F32, tag="vtr")
                        nc.tensor.transpose(
                            psv[:], vT[pp_][:, tt * P:(tt + 1) * P], ident[:])
                        nc.vector.tensor_copy(
                            v_sbp[pp_][:, tt, 0:DK], psv[:, 0:DK])
                        nc.vector.tensor_copy(
                            v_sbp[pp_][:, tt, 2 * DK:3 * DK], psv[:, DK:2 * DK])

            # ---- stage C tiles allocated now (reuse stage A's space);
            # the 4 MB wo load overlaps stage B compute on the sync queue
            with (
                tc.tile_pool(name="stageC", bufs=1) as c2pool,
                tc.tile_pool(name="yout", bufs=2) as ypool,
            ):
                wo_sb = c2pool.tile([P, 8, D], F32R)
                nc.sync.dma_start(wo_sb[:], woT[:].rearrange("(o p) n -> p o n", p=P))
                h_sb = c2pool.tile([P, 2, 8, RB], F32R)

                # ---- stage B: attention per output i-chunk ----
                a2a_in = dpool.tile([NCORES, CS, RB], F32R)
                a2a_out = dpool.tile([NCORES, CS, RB], F32R)
                with (
                    tc.tile_pool(name="et", bufs=7) as etpool,
                    tc.tile_pool(name="ob", bufs=3) as obpool,
                    tc.tile_pool(name="psS", bufs=3, space="PSUM") as psS,
                    tc.tile_pool(name="psAV", bufs=1, space="PSUM") as psAV,
                ):
                    for g in range(S // IB):
                        i0 = g * IB
                        av_ps = [
                            psAV.tile([P, IB], F32, tag=f"av{h % 2}",
                                      name=f"av{h}")
                            for h in range(HPC)
                        ]
                        for pp_ in range(NPAIR):
                            for jt in range(S // P):
                                j0 = jt * P
                                sps = psS.tile([P, 2 * IB], F32, tag="s")
                                for hh in range(2):
                                    nc.tensor.matmul(
                                        sps[:, hh * IB:(hh + 1) * IB],
                                        kTh[2 * pp_ + hh][:, j0:j0 + P],
                                        qTq[pp_][:, i0:i0 + IB],
                                        start=True,
                                        stop=True,
                                    )
                                et = etpool.tile([P, 2 * IB], F32R, tag="et")
                                nc.scalar.activation(
                                    et[:], sps[:], AF.Exp, scale=0.125)
                                for hh in range(2):
                                    nc.tensor.matmul(
                                        av_ps[2 * pp_ + hh][:],
                                        v_sbp[pp_][:, jt, hh * DK:hh * DK + 2 * DK],
                                        et[:, hh * IB:(hh + 1) * IB],
                                        start=(jt == 0),
                                        stop=(jt == S // P - 1),
                                    )
                        for pp_ in range(NPAIR):
                            # merge the pair: raw rows into A, denom rows into
                            # D (even head: raw 0:64/den 64:128; odd head
                            # flipped), one reciprocal + multiply per pair
                            e, o = 2 * pp_, 2 * pp_ + 1
                            rawA = obpool.tile([P, IB], F32, tag="rawA")
                            nc.vector.tensor_copy(rawA[0:DK, :], av_ps[e][0:DK, :])
                            nc.vector.tensor_copy(rawA[DK:, :], av_ps[o][DK:, :])
                            den = obpool.tile([P, IB], F32, tag="den")
                            nc.vector.tensor_copy(den[0:DK, :], av_ps[e][DK:, :])
                            nc.vector.tensor_copy(den[DK:, :], av_ps[o][0:DK, :])
                            rec = obpool.tile([P, IB], F32, tag="rec")
                            nc.vector.reciprocal(rec[:], den[:])
                            onrm = obpool.tile([P, IB], F32, tag="onrm")
                            nc.vector.tensor_tensor(
                                onrm[:], rawA[:], rec[:], ALU.mult,
                            )
                            # chunk rows [512g:512g+512) split to dests 2g,2g+1
                            for dd2 in range(2):
                                nc.sync.dma_start(
                                    a2a_in[2 * g + dd2,
                                           pp_ * P:(pp_ + 1) * P, :],
                                    onrm[:, dd2 * RB:(dd2 + 1) * RB
                                         ].bitcast(F32R),
                                )

                nc.gpsimd.collective_compute(
                    "AllToAll",
                    ALU.bypass,
                    replica_groups=[list(range(NCORES))],
                    ins=[a2a_in.opt()],
                    outs=[a2a_out.opt()],
                )

                # ---- stage C: output projection for my 2x256 rows ----
                # received chunk s (s<4: batch0, s>=4: batch1) holds channels
                # [256s mod 1024 ...] of my row block; channel sub-tile o of
                # half hb lives at a2a_out[hb*4 + o//2, (o%2)*128:...]
                with tc.tile_pool(name="psY", bufs=2, space="PSUM") as psY:
                    for hb in range(2):
                        for o in range(8):
                            nc.sync.dma_start(
                                h_sb[:, hb, o, :],
                                a2a_out[hb * GPB + o // 2,
                                        (o % 2) * P:(o % 2 + 1) * P, :],
                            )
                    for hb in range(2):
                        for it in range(RB // P):
                            y_sb = ypool.tile([P, D], F32, tag="y")
                            for nch in range(D // 512):
                                py = psY.tile([P, 512], F32, tag="py")
                                for o in range(8):
                                    nc.tensor.matmul(
                                        py[:],
                                        h_sb[:, hb, o, it * P:(it + 1) * P],
                                        wo_sb[:, o, nch * 512:(nch + 1) * 512],
                                        start=(o == 0),
                                        stop=(o == 7),
                                    )
                                nc.vector.tensor_tensor(
                                    y_sb[:, nch * 512:(nch + 1) * 512],
                                    py[:],
                                    bo_full[:, nch * 512:(nch + 1) * 512],
                                    ALU.add,
                                )
                            r0 = hb * RB + it * P
                            nc.sync.dma_start(y[r0:r0 + P, :], y_sb[:])

    nc.compile()
    return nc


_NC = None


def _get_nc():
    global _NC
    if _NC is None:
        _NC = build_nc()
    return _NC


def _make_in_maps(x, Wq, bq, Wk, bk, Wv, bv, Wo, bo):
    xf = np.asarray(x, np.float32).reshape(B, S, D)
    woT = np.ascontiguousarray(np.asarray(Wo, np.float32).T)
    bo_r = np.ascontiguousarray(np.asarray(bo, np.float32).reshape(1, D))
    Wq = np.asarray(Wq, np.float32)
    Wk = np.asarray(Wk, np.float32)
    Wv = np.asarray(Wv, np.float32)
    in_maps = []
    for c in range(NCORES):
        b = c // GPB
        r = c % GPB
        sl = slice(r * CS, (r + 1) * CS)
        in_maps.append({
            "x": np.ascontiguousarray(xf[b]),
            "wqT": np.ascontiguousarray(Wq[sl, :].T),
            "wkT": np.ascontiguousarray(Wk[sl, :].T),
            "wvT": np.ascontiguousarray(Wv[sl, :].T),
            "woT": woT,
            "bq": np.ascontiguousarray(
                np.asarray(bq, np.float32)[sl].reshape(NPAIR, P).T),
            "bk": np.ascontiguousarray(
                np.asarray(bk, np.float32)[sl].reshape(NPAIR, P).T),
            "bv": np.ascontiguousarray(
                np.asarray(bv, np.float32)[sl].reshape(NPAIR, P).T),
            "bo": bo_r,
        })
    return in_maps


def _assemble(results):
    yout = np.empty((B, S, D), np.float32)
    for d in range(NCORES):
        rows = slice(d * RB, (d + 1) * RB)
        yout[0, rows, :] = results[d]["y"][0:RB]
        yout[1, rows, :] = results[d]["y"][RB:2 * RB]
    return yout


def run_traced(trace=False, **inputs):
    """Run and return (output, BassKernelResults) — used by test.py."""
    nc = _get_nc()
    res = run_bass_kernel_spmd(
        nc, _make_in_maps(**inputs), core_ids=list(range(NCORES)), trace=trace
    )
    return _assemble(res.results), res


def kernel(**inputs) -> np.ndarray:
    out, _ = run_traced(trace=False, **inputs)
    return out
